# revision 2
# baseline (speedup 1.0000x reference)
"""Trainium2 Bass kernel for nn_Attention_67336497266780 — v2.

Single-head attention, B=8 S=2048 E=1024 H=64, data-parallel over batch:
each of the 8 NeuronCores computes one batch element end to end.

v2 changes over the baseline (101.7us/iter -> target ~75us, DMA floor
~71-75us for the 25.2MB of f32 q/k/v HBM reads per core):
  - Fused transpose+projection pipeline: PE alternates chunk-pair
    transposes with the previous pair's projection matmuls, so the PE no
    longer stalls at drain (PSUM->SBUF) pace.
  - Paired drains ([128,2,512] = one PSUM bank per 2 e-chunks), spread
    over ACT+DVE in the q phase and DVE in the k/v phase.
  - bk dropped: softmax is exactly invariant to the per-query score
    shift q.bk it induces.
  - qt written once per block into the partition half that the score
    matmuls actually read (baseline wrote both halves); kt scatter-
    copied to both halves (stationary needs both row groups).
  - PV in fp8e4m3 with MatmulPerfMode.DoubleRow (two k-tiles per pass,
    0.5 cyc/row): exp_all and v' stored fp8; ~24k PE cycles saved.
  - Scores for k-tiles m2/m3 of block b deferred into block b+1's
    PE gap (while waiting for the v DMA), keeping PE busy.
  - Output DMA issued from SP (idle queue).

Self-contained: hardcodes shapes; builds + compiles once per process and
caches the jitted PJRT executable for subsequent calls.
"""
import sys

try:
    import concourse  # noqa: F401  (resolves via PYTHONPATH when present)
except ImportError:
    sys.path.insert(0, "/opt/trn_rl_repo")

from contextlib import ExitStack

import numpy as np

import concourse.bass as bass  # noqa: F401
import concourse.mybir as mybir
import concourse.tile as tile
from concourse import bacc
from concourse.masks import make_identity

F32 = mybir.dt.float32
BF16 = mybir.dt.bfloat16
FP8 = mybir.dt.float8e4
U32 = mybir.dt.uint32

B = 8
P = 128
S = 2048
E = 1024
H = 64
EC = E // P          # 8 e-chunks
ST = S // P          # 16 s-tiles
NBLK = 4             # 4 blocks of 512 rows
BT = 4               # s-tiles per block
SBLK = BT * P        # 512
QCH = 512            # q-chunk width

_CACHE = {}


def build(repeat=0, debug=False, unroll=0, fp8_pv=True, all_hints=True,
          pv_head=True, xv_bufs=4, xt_bufs=3, tr_bufs=3):
    nc = bacc.Bacc("TRN2", target_bir_lowering=False, debug=debug)

    xq_ext = nc.dram_tensor("query", [S, E], F32, kind="ExternalInput")
    xk_ext = nc.dram_tensor("key", [S, E], F32, kind="ExternalInput")
    xv_ext = nc.dram_tensor("value", [S, E], F32, kind="ExternalInput")
    wq_ext = nc.dram_tensor("Wq", [E, H], F32, kind="ExternalInput")
    wk_ext = nc.dram_tensor("Wk", [E, H], F32, kind="ExternalInput")
    wv_ext = nc.dram_tensor("Wv", [E, H], F32, kind="ExternalInput")
    bq_ext = nc.dram_tensor("bq", [H], F32, kind="ExternalInput")
    bk_ext = nc.dram_tensor("bk", [H], F32, kind="ExternalInput")  # unused
    bv_ext = nc.dram_tensor("bv", [H], F32, kind="ExternalInput")
    out_ext = nc.dram_tensor("out", [S, H], F32, kind="ExternalOutput")

    EXPD = FP8 if fp8_pv else BF16
    Act = mybir.ActivationFunctionType

    ctx = ExitStack()
    with tile.TileContext(nc) as tc, ctx:
        const = ctx.enter_context(tc.tile_pool(name="const", bufs=1))
        persist = ctx.enter_context(tc.tile_pool(name="persist", bufs=1))
        xv_pool = ctx.enter_context(tc.tile_pool(name="xv", bufs=xv_bufs))
        xt_pool = ctx.enter_context(tc.tile_pool(name="xt", bufs=xt_bufs))
        outt_pool = ctx.enter_context(tc.tile_pool(name="outt", bufs=2))
        ps_tr = ctx.enter_context(
            tc.tile_pool(name="ps_tr", bufs=tr_bufs, space="PSUM"))
        ps_proj = ctx.enter_context(
            tc.tile_pool(name="ps_proj", bufs=1, space="PSUM"))
        ps_sc = ctx.enter_context(
            tc.tile_pool(name="ps_sc", bufs=2, space="PSUM"))

        # ---- constants (outside timing loop) ----
        ident = const.tile([P, P], BF16, name="ident")
        make_identity(nc, ident)
        ident_f = const.tile([P, P], F32, name="ident_f")
        make_identity(nc, ident_f)

        w_sb = {}
        b_sb = {}
        for name, wext, bext in (("q", wq_ext, bq_ext), ("v", wv_ext, bv_ext),
                                 ("k", wk_ext, None)):
            w_raw = const.tile([P, EC, H], F32, name=f"wraw{name}")
            nc.scalar.dma_start(w_raw[:], wext.rearrange("(o p) h -> p o h", p=P))
            w = const.tile([P, EC, H], BF16, name=f"w{name}")
            nc.scalar.copy(out=w[:], in_=w_raw[:])
            w_sb[name] = w
            if bext is not None:
                b = const.tile([H, 1], F32, name=f"b{name}")
                nc.scalar.dma_start(b[:], bext[:].unsqueeze(1))
                b_sb[name] = b

        qt_sb = persist.tile([P, S], BF16, name="qt")     # halves by chunk parity
        kt_sb = persist.tile([P, S], BF16, name="kt")     # dup halves
        vt_sb = persist.tile([H, S], BF16, name="vt")
        vp_sb = persist.tile([P, ST, H + 2], EXPD, name="vprime")  # col H=1s
        exp_all = persist.tile([P, ST, S], EXPD, name="exp_all")
        out_sb = persist.tile([P, ST, H], F32, name="out_sb")
        rc_sb = persist.tile([P, ST], F32, name="rc")

        # init so iteration 0's PV-head reads defined data
        nc.vector.memset(vp_sb[:], 1.0)
        if repeat or unroll > 1:
            nc.gpsimd.memset(exp_all[:], 1.0)

        if repeat:
            if all_hints:
                hints = (mybir.EngineType.PE, mybir.EngineType.DVE,
                         mybir.EngineType.Activation, mybir.EngineType.SP,
                         mybir.EngineType.Pool)
            else:
                hints = (mybir.EngineType.PE, mybir.EngineType.DVE)
            loop_cm = tc.For_i(0, repeat, 1, hint_engines=hints)
        else:
            import contextlib
            loop_cm = contextlib.nullcontext()

        def load_block(xext, b):
            """SWDGE cast-DMA f32->bf16.  Partition p holds block rows
            4p..4p+3 (16KB contiguous DRAM per partition -> 128 descs)."""
            x_t = xv_pool.tile([P, BT, E], BF16, tag="xv")
            src = xext[b * SBLK:(b + 1) * SBLK, :].rearrange(
                "(p t) e -> p t e", p=P)
            nc.gpsimd.dma_start(x_t[:], src)
            return x_t

        def trans_proj(x_t, tag, drains, fill3=None):
            """Fused pipeline over 4 chunk-pairs: transpose 8 tiles of
            pair cp into one PSUM bank, drain pair (ACT or DVE), then the
            previous pair's projection matmuls on PE.  fill3 emits extra
            PE work after pair 3's transposes (covers its drain latency)."""
            xt_t = xt_pool.tile([P, EC, SBLK], BF16, tag="xt")
            proj_ps = ps_proj.tile([H, SBLK], F32, tag="proj")
            w = w_sb[tag]

            def proj_pair(cp):
                for c in (2 * cp, 2 * cp + 1):
                    nc.tensor.matmul(
                        proj_ps[:], lhsT=w[:, c], rhs=xt_t[:, c],
                        start=(c == 0), stop=(c == EC - 1))

            for cp in range(EC // 2):
                t_ps = ps_tr.tile([P, 2, SBLK], BF16, tag="tr")
                for half in (0, 1):
                    c = 2 * cp + half
                    for t in range(BT):
                        nc.tensor.transpose(
                            t_ps[:, half, t * P:(t + 1) * P],
                            x_t[:, t, c * P:(c + 1) * P],
                            ident)
                nc.vector.tensor_copy(
                    out=xt_t[:, 2 * cp:2 * cp + 2].bitcast(U32),
                    in_=t_ps[:].bitcast(U32))
                if cp == 3 and fill3 is not None:
                    fill3()
                if cp >= 1:
                    proj_pair(cp - 1)
            proj_pair(3)
            return proj_ps

        def finish_q(proj_ps, b):
            """bias-add into the partition half the scores read chunk b
            from (ACT; q phase has ACT headroom)."""
            half = b % 2
            sl = slice(b * SBLK, (b + 1) * SBLK)
            nc.scalar.activation(
                qt_sb[half * H:(half + 1) * H, sl], proj_ps[:],
                Act.Identity, bias=b_sb["q"], scale=1.0)

        def finish_k(proj_ps, b):
            """kt is stationary for both row groups -> dup both halves.
            bk dropped (softmax shift-invariance)."""
            sl = slice(b * SBLK, (b + 1) * SBLK)
            for half in (0, 1):
                nc.vector.tensor_copy(
                    out=kt_sb[half * H:(half + 1) * H, sl], in_=proj_ps[:])

        def finish_v(proj_ps, b):
            sl = slice(b * SBLK, (b + 1) * SBLK)
            nc.vector.tensor_scalar(
                out=vt_sb[:, sl], in0=proj_ps[:], scalar1=b_sb["v"],
                scalar2=None, op0=mybir.AluOpType.add)

        def vprime_block(b):
            """vt block -> vp[:, m, 0:H] for the block's 4 m-tiles."""
            t_ps = ps_tr.tile([P, BT * H], BF16, tag="tr")
            for t in range(BT):
                m = b * BT + t
                nc.tensor.transpose(
                    t_ps[:, t * H:(t + 1) * H],
                    vt_sb[:, m * P:(m + 1) * P],
                    ident[:H, :H])
            nc.vector.tensor_copy(
                out=vp_sb[:, b * BT:(b + 1) * BT, 0:H],
                in_=t_ps[:].rearrange("p (t h) -> p t h", t=BT))

        def scores_ktile(m):
            """scores^T for k-tile m: 4 matmuls, 2 LDWs (row-group packed),
            two [128,1024] PSUM tiles; exp 1024-wide -> exp_all (fp8)."""
            mc = slice(m * P, (m + 1) * P)
            scA = ps_sc.tile([P, 2 * QCH], F32, tag="sc")
            scB = ps_sc.tile([P, 2 * QCH], F32, tag="sc")
            # row-group 0 (partitions 0:64): q chunks 0 and 2
            nc.tensor.matmul(scA[:, 0:QCH], lhsT=kt_sb[0:H, mc],
                             rhs=qt_sb[0:H, 0:QCH], start=True, stop=True)
            nc.tensor.matmul(scB[:, 0:QCH], lhsT=kt_sb[0:H, mc],
                             rhs=qt_sb[0:H, 2 * QCH:3 * QCH],
                             start=True, stop=True)
            # row-group 1 (partitions 64:128): q chunks 1 and 3
            nc.tensor.matmul(scA[:, QCH:2 * QCH], lhsT=kt_sb[H:2 * H, mc],
                             rhs=qt_sb[H:2 * H, QCH:2 * QCH],
                             start=True, stop=True)
            nc.tensor.matmul(scB[:, QCH:2 * QCH], lhsT=kt_sb[H:2 * H, mc],
                             rhs=qt_sb[H:2 * H, 3 * QCH:4 * QCH],
                             start=True, stop=True)
            nc.scalar.activation(
                exp_all[:, m, 0:2 * QCH], scA[:], Act.Exp, scale=0.125)
            nc.scalar.activation(
                exp_all[:, m, 2 * QCH:4 * QCH], scB[:], Act.Exp, scale=0.125)

        def pv_block(b):
            """outT[65, 512] for q-chunk b; transpose back, normalize,
            DMA out (SP queue)."""
            pv = ps_sc.tile([H + 1, QCH], F32, tag="sc")
            if fp8_pv:
                for m2 in range(ST // 2):
                    nc.tensor.matmul(
                        pv[:], lhsT=vp_sb[:, 2 * m2:2 * m2 + 2, 0:H + 1],
                        rhs=exp_all[:, 2 * m2:2 * m2 + 2,
                                    b * QCH:(b + 1) * QCH],
                        start=(m2 == 0), stop=(m2 == ST // 2 - 1),
                        perf_mode=mybir.MatmulPerfMode.DoubleRow)
            else:
                for m in range(ST):
                    nc.tensor.matmul(
                        pv[:], lhsT=vp_sb[:, m, 0:H + 1],
                        rhs=exp_all[:, m, b * QCH:(b + 1) * QCH],
                        start=(m == 0), stop=(m == ST - 1))
            outt = outt_pool.tile([H + 1, QCH], F32, tag="outt")
            nc.vector.tensor_copy(out=outt[:], in_=pv[:])
            for t in range(BT):
                qt_idx = b * BT + t
                o_ps = ps_tr.tile([P, H + 1], F32, tag="tr")
                nc.tensor.transpose(
                    o_ps[:],
                    outt[:, t * P:(t + 1) * P],
                    ident_f[:H + 1, :H + 1])
                nc.vector.reciprocal(rc_sb[:, qt_idx:qt_idx + 1],
                                     o_ps[:, H:H + 1])
                nc.vector.tensor_scalar(
                    out=out_sb[:, qt_idx], in0=o_ps[:, 0:H],
                    scalar1=rc_sb[:, qt_idx:qt_idx + 1],
                    scalar2=None, op0=mybir.AluOpType.mult)
            nc.sync.dma_start(
                out_ext[b * SBLK:(b + 1) * SBLK, :].rearrange(
                    "(p t) h -> p t h", p=P),
                out_sb[:, b * BT:(b + 1) * BT])

        def body(with_pv_head):
            # ---- phase Q (+ PV head of previous iteration) ----
            for b in range(NBLK):
                xb = load_block(xq_ext, b)
                if with_pv_head:
                    pv_block(b)
                pp = trans_proj(xb, "q", drains=("act", "dve", "act", "dve"))
                finish_q(pp, b)
            # ---- phase K/V + scores + exp ----
            # k-tiles m2/m3 of block b run in block b+1's k->v DMA gap;
            # m0 fills pair-3 of the v transpose pipeline, m1 the block
            # tail.  Block 3 runs its own m2/m3 in-block.
            for b in range(NBLK):
                xkb = load_block(xk_ext, b)
                ppk = trans_proj(xkb, "k", drains=("dve",) * 4)
                finish_k(ppk, b)
                if b > 0:
                    scores_ktile((b - 1) * BT + 2)
                    scores_ktile((b - 1) * BT + 3)
                xvb = load_block(xv_ext, b)
                ppv = trans_proj(
                    xvb, "v", drains=("dve",) * 4,
                    fill3=lambda b=b: scores_ktile(b * BT + 0))
                finish_v(ppv, b)
                vprime_block(b)
                scores_ktile(b * BT + 1)
                if b == NBLK - 1:
                    scores_ktile(b * BT + 2)
                    scores_ktile(b * BT + 3)

        if unroll:
            for i in range(unroll):
                body(with_pv_head=(i > 0))
        else:
            with loop_cm:
                body(with_pv_head=bool(repeat) and pv_head)

        # ---- tail: final PV + output ----
        for b in range(NBLK):
            pv_block(b)

    nc.compile()
    return nc


def _get_runner():
    if "runner" in _CACHE:
        return _CACHE["runner"]

    import functools
    import traceback

    import jax
    from jax.experimental.shard_map import shard_map
    from jax.sharding import Mesh, PartitionSpec

    from concourse import bass2jax
    from concourse.bass2jax import _bass_exec_p, partition_id_tensor

    bass2jax.install_neuronx_cc_hook()
    import libneuronxla
    hook = libneuronxla.neuronx_cc
    if not getattr(hook, "_verbose_wrapped", False):
        @functools.wraps(hook)
        def wrapped(*a, **k):
            try:
                return hook(*a, **k)
            except BaseException:
                traceback.print_exc()
                sys.stderr.flush()
                raise
        wrapped._verbose_wrapped = True
        libneuronxla.neuronx_cc = wrapped

    nc = build()

    partition_name = nc.partition_id_tensor.name if nc.partition_id_tensor else None
    in_names, out_names, out_avals, zero_outs = [], [], [], []
    for alloc in nc.m.functions[0].allocations:
        if not isinstance(alloc, mybir.MemoryLocationSet):
            continue
        name = alloc.memorylocations[0].name
        if alloc.kind == "ExternalInput":
            if name != partition_name:
                in_names.append(name)
        elif alloc.kind == "ExternalOutput":
            out_names.append(name)
            shape = tuple(alloc.tensor_shape)
            dtype = mybir.dt.np(alloc.dtype)
            out_avals.append(jax.core.ShapedArray(shape, dtype))
            zero_outs.append(np.zeros(shape, dtype))
    n_params = len(in_names)
    n_outs = len(out_avals)
    all_in_names = list(in_names) + out_names
    if partition_name is not None:
        all_in_names.append(partition_name)
    donate = tuple(range(n_params, n_params + n_outs))

    def _body(*args):
        operands = list(args)
        if partition_name is not None:
            operands.append(partition_id_tensor())
        outs = _bass_exec_p.bind(
            *operands,
            out_avals=tuple(out_avals),
            in_names=tuple(all_in_names),
            out_names=tuple(out_names),
            lowering_input_output_aliases=(),
            sim_require_finite=False,
            sim_require_nnan=False,
            nc=nc,
        )
        return tuple(outs)

    devices = jax.devices()[:B]
    mesh = Mesh(np.asarray(devices), ("core",))
    in_specs = (PartitionSpec("core"),) * (n_params + n_outs)
    out_specs = (PartitionSpec("core"),) * len(out_names)
    sharded = jax.jit(
        shard_map(_body, mesh=mesh, in_specs=in_specs,
                  out_specs=out_specs, check_rep=False),
        donate_argnums=donate, keep_unused=True)

    runner = {
        "sharded": sharded, "in_names": in_names, "out_names": out_names,
        "out_avals": out_avals, "zero_outs": zero_outs,
    }
    _CACHE["runner"] = runner
    return runner


def kernel(**inputs):
    r = _get_runner()
    per_core = {"query", "key", "value"}

    concat_in = []
    for name in r["in_names"]:
        arr = np.ascontiguousarray(np.asarray(inputs[name], dtype=np.float32))
        if name in per_core:
            concat_in.append(arr.reshape(B * S, E))
        else:
            concat_in.append(np.concatenate([arr] * B, axis=0))
    concat_zeros = [
        np.zeros((B * z.shape[0], *z.shape[1:]), z.dtype) for z in r["zero_outs"]
    ]
    out_arrs = r["sharded"](*concat_in, *concat_zeros)
    (aval,) = r["out_avals"]
    out = np.asarray(out_arrs[0]).reshape(B, *aval.shape)
    return out.astype(np.float32, copy=False)


if __name__ == "__main__":
    rng = np.random.default_rng(0)
    fake = {
        "query": rng.standard_normal((B, S, E), dtype=np.float32),
        "key": rng.standard_normal((B, S, E), dtype=np.float32),
        "value": rng.standard_normal((B, S, E), dtype=np.float32),
        "Wq": rng.standard_normal((E, H), dtype=np.float32) / 32,
        "bq": np.zeros(H, np.float32),
        "Wk": rng.standard_normal((E, H), dtype=np.float32) / 32,
        "bk": np.zeros(H, np.float32),
        "Wv": rng.standard_normal((E, H), dtype=np.float32) / 32,
        "bv": np.zeros(H, np.float32),
    }
    out = kernel(**fake)
    print("kernel out:", out.shape, out.dtype, float(out[0, 0, 0]))
